# revision 2
# baseline (speedup 1.0000x reference)
"""Trainium2 Bass kernel for the one-hot Conv2DProduct.

Math: the reference is a VALID conv, stride (2,2), kernel 2x2, with a one-hot
HWIO weight where output channel o selects input channel (o // 32**k) % 32 at
kernel cell k (row-major cells).  With C_OUT = 512 < 32**2, cells 2 and 3
always select channel 0, so

  out[b, i, j, o] = x[b, 2i, 2j,   o % 32]      (cell 0: even row, even col)
                  + x[b, 2i, 2j+1, o // 32]     (cell 1: even row, odd col; o//32 < 16)
                  + x[b, 2i+1, 2j,   0]         (cell 2)
                  + x[b, 2i+1, 2j+1, 0]         (cell 3)

i.e. per output pixel an outer sum over (c1, c0) = (o//32, o%32) plus a
per-pixel scalar.  The kernel computes, per core (8 batches):

  s[p, j]        = x_odd[p, 64j] + x_odd[p, 64j + 32]          (channel 0 of both odd-row pixels)
  Bs[p, j, c1]   = x_even[p, 64j + 32 + c1] + s[p, j]          (c1 < 16)
  out[p, j, c1, c0] = x_even[p, 64j + c0] + Bs[p, j, c1]

with SBUF partition p = (batch_pair, output row i), everything done with three
broadcast tensor_tensor adds on the vector engine.  Data-parallel over batch
across the 8 cores.
"""

import sys

import numpy as np

_REPO = "/opt/trn_rl_repo"
if _REPO not in sys.path:
    sys.path.insert(0, _REPO)

import concourse.bacc as bacc
import concourse.mybir as mybir
from concourse import tile
from concourse.bass_utils import run_bass_kernel_spmd

B, H, W, C = 64, 128, 128, 32
OH, OW, CO = 64, 64, 512
N_CORES = 8
B_LOC = B // N_CORES  # batches per core
F32 = mybir.dt.float32


def build_bass(b_loc: int = B_LOC):
    nc = bacc.Bacc("TRN2", target_bir_lowering=False, debug=False)
    x = nc.dram_tensor("x", [b_loc, H, W, C], F32, kind="ExternalInput")
    out = nc.dram_tensor("out", [b_loc, OH, OW, CO], F32, kind="ExternalOutput")

    with tile.TileContext(nc) as tc:
        with (
            tc.tile_pool(name="io", bufs=2) as io_pool,
            tc.tile_pool(name="mid", bufs=2) as mid_pool,
            tc.tile_pool(name="outp", bufs=3) as out_pool,
        ):
            # x rows split even/odd; partition dim = (batch, out-row i).
            x_r = x[:].rearrange("b (i two) w c -> two (b i) (w c)", two=2)
            out_d = out[:].rearrange("b i j o -> (b i) (j o)")

            n_bg = (b_loc * OH) // 128  # batch-groups of 128 partitions
            for bg in range(n_bg):
                psl = slice(bg * 128, (bg + 1) * 128)
                xe = io_pool.tile([128, W * C], F32, name=f"xe{bg}", tag="xe")
                xo = io_pool.tile([128, W * C], F32, name=f"xo{bg}", tag="xo")
                nc.sync.dma_start(xe[:], x_r[0][psl, :])
                nc.sync.dma_start(xo[:], x_r[1][psl, :])

                xe_r = xe.rearrange("p (j two c) -> p j two c", two=2, c=C)
                xo_r = xo.rearrange("p (j two c) -> p j two c", two=2, c=C)

                s = mid_pool.tile([128, OW], F32, name=f"s{bg}", tag="s")
                nc.vector.tensor_tensor(
                    out=s[:],
                    in0=xo_r[:, :, 0, 0],
                    in1=xo_r[:, :, 1, 0],
                    op=mybir.AluOpType.add,
                )

                bs = mid_pool.tile([128, OW * 16], F32, name=f"bs{bg}", tag="bs")
                nc.vector.tensor_tensor(
                    out=bs[:],
                    in0=xe_r[:, :, 1, 0:16],
                    in1=s[:].unsqueeze(2).to_broadcast([128, OW, 16]),
                    op=mybir.AluOpType.add,
                )
                bs_r = bs.rearrange("p (j c1) -> p j c1", c1=16)

                for jc in range(4):
                    jsl = slice(jc * 16, (jc + 1) * 16)
                    ot = out_pool.tile([128, 16 * CO], F32, name=f"ot{bg}_{jc}", tag="ot")
                    nc.vector.tensor_tensor(
                        out=ot[:],
                        in0=xe_r[:, jsl, 0, :].unsqueeze(2).to_broadcast([128, 16, 16, C]),
                        in1=bs_r[:, jsl, :].unsqueeze(3).to_broadcast([128, 16, 16, C]),
                        op=mybir.AluOpType.add,
                    )
                    nc.sync.dma_start(
                        out_d[psl, jc * 16 * CO:(jc + 1) * 16 * CO], ot[:]
                    )
    return nc


_NC = None


def _get_nc():
    global _NC
    if _NC is None:
        _NC = build_bass()
        _NC.compile()  # bacc register allocation + lowering
    return _NC


def kernel(**inputs):
    x = np.ascontiguousarray(np.asarray(inputs["x"], dtype=np.float32))
    assert x.shape == (B, H, W, C), x.shape
    nc = _get_nc()
    in_maps = [
        {"x": np.ascontiguousarray(x[c * B_LOC:(c + 1) * B_LOC])}
        for c in range(N_CORES)
    ]
    res = run_bass_kernel_spmd(nc, in_maps, list(range(N_CORES))).results
    return np.concatenate([np.asarray(r["out"]) for r in res], axis=0)
